# revision 2
# baseline (speedup 1.0000x reference)
"""Multi-Head Latent Attention (MLA) Trainium2 kernel, 8-way sharded.

Sharding: 8 cores = 2 (batch) x 4 (head groups of 4 heads).
Each core handles one batch element and 4 of the 16 heads.

v2 changes vs baseline:
  - Query path folded on host: q[:, base|rope] = x @ (W_D_Q @ [W_U_Q_g | W_Q_R_g]),
    eliminating the duplicated qc = x @ W_D_Q latent (DCQ=1536) per core.
  - All projection weights SBUF-resident, loaded once (baseline re-streamed
    them every token chunk: ~104MB -> ~18MB weight traffic per core).
  - q/k/v spills to DRAM in bf16 (half traffic); attention matmuls run in
    bf16 (same PE rate as f32r: 1 cyc/row), accumulating in f32 PSUM.
  - Softmax denominators: P^T tiles accumulated on DVE into pt_acc, then ONE
    ones-matmul per (head, query-block) instead of one per key-tile
    (saves ~123k PE cycles/core).

Everything is computed TRANSPOSED (feature dim on partitions), as in v1:
scores come out as S^T (keys on partitions, queries free), softmax = plain
exp (scores O(+-6)), no on-chip transposes; RoPE pair-swap is a small
constant matmul plus elementwise mul/add.
"""

import sys

sys.path.insert(0, "/opt/trn_rl_repo")

import numpy as np

import concourse.bacc as bacc
import concourse.mybir as mybir
import concourse.tile as tile
from concourse.bass_utils import run_bass_kernel_spmd

# Problem dims (hardcoded per contract)
D, NH, DH, DC, DCQ, DHR = 2048, 16, 128, 512, 1536, 64
B, L = 2, 2048
ROPE_THETA = 10000.0

NHG = 4                 # heads per core
DQB = NHG * DH          # 512: per-core base q/k feature dim (also v dim)
DQR = NHG * DHR         # 256: per-core rope feature dim
DQ = DQB + DQR          # 768: folded q feature dim
P = 128
CW = 256                # phase-A token chunk width
SCALE = DH ** -0.5

F32R = mybir.dt.float32r
F32 = mybir.dt.float32
BF16 = mybir.dt.bfloat16

_CACHED = {}


def _build(repeat=None):
    """Build the SPMD program. repeat=N wraps the body in a HW loop (for
    perf measurement only — amortizes host dispatch overhead)."""
    nc = bacc.Bacc("TRN2", target_bir_lowering=False, debug=False)

    # ---- DRAM I/O (per-core data; program is SPMD)
    xT = nc.dram_tensor("xT", [D, L], F32R, kind="ExternalInput")
    weff = nc.dram_tensor("weff", [D, DQ], F32R, kind="ExternalInput")
    wdkv = nc.dram_tensor("wdkv", [D, DC], F32R, kind="ExternalInput")
    wuk = nc.dram_tensor("wuk", [DC, DQB], F32R, kind="ExternalInput")
    wkr = nc.dram_tensor("wkr", [D, DQR], F32R, kind="ExternalInput")
    wuv = nc.dram_tensor("wuv", [DC, DQB], F32R, kind="ExternalInput")
    wo = nc.dram_tensor("wo", [DQB, D], F32R, kind="ExternalInput")
    cosr = nc.dram_tensor("cosr", [P, L], F32, kind="ExternalInput")
    sinr = nc.dram_tensor("sinr", [P, L], F32, kind="ExternalInput")
    protT = nc.dram_tensor("protT", [P, P], F32R, kind="ExternalInput")
    onesd = nc.dram_tensor("onesd", [P, P], F32R, kind="ExternalInput")
    out = nc.dram_tensor("out", [L, D], F32, kind="ExternalOutput")

    # ---- internal DRAM spill (transposed q/k, natural v) — bf16
    qbT_d = nc.dram_tensor("qbT_d", [DQB, L], BF16)
    qrT_d = nc.dram_tensor("qrT_d", [DQR, L], BF16)
    kbT_d = nc.dram_tensor("kbT_d", [DQB, L], BF16)
    krT_d = nc.dram_tensor("krT_d", [DQR, L], BF16)
    v_d = nc.dram_tensor("v_d", [L, DQB], BF16)

    KD = D // P      # 16
    KC = DC // P     # 4
    NCH = L // CW    # chunks

    def load_wcols(pool, w_src, nrows, ncols, wtag):
        """Load [nrows, ncols] weight as ncols//P column-tiles, each a
        [P, nk, P] 3-D tile (one batched DMA per column tile)."""
        nk = nrows // P
        tiles = []
        for m in range(ncols // P):
            wt = pool.tile([P, nk, P], F32R, name=f"w_{wtag}{m}", tag=f"w_{wtag}{m}")
            src = w_src[:, m * P:(m + 1) * P].rearrange("(k p) j -> p k j", p=P)
            nc.sync.dma_start(out=wt[:], in_=src)
            tiles.append(wt)
        return tiles

    def mm_acc(ps, wt, rhs_tiles, nk):
        for k in range(nk):
            nc.tensor.matmul(ps[:], wt[:, k, :], rhs_tiles[k][:],
                             start=(k == 0), stop=(k == nk - 1))

    from contextlib import nullcontext
    with tile.TileContext(nc) as tc:
        with (tc.For_i(0, repeat, 1) if repeat else nullcontext()), \
             tc.tile_pool(name="constp", bufs=1) as constp, \
             tc.tile_pool(name="otp_res", bufs=1) as otp_res:
            prot_t = constp.tile([P, P], F32R, name="prot_t", tag="prot")
            nc.sync.dma_start(out=prot_t[:], in_=protT[:, :])
            ones_t = constp.tile([P, P], F32R, name="ones_t", tag="ones")
            nc.sync.dma_start(out=ones_t[:], in_=onesd[:, :])
            oT_res = [otp_res.tile([P, L], F32R, name=f"oT{h}", tag=f"oT{h}")
                      for h in range(NHG)]

            # ================= Phase A: projections (token-chunked) =========
            with tc.tile_pool(name="wres", bufs=1) as wres, \
                 tc.tile_pool(name="xp", bufs=24) as xp, \
                 tc.tile_pool(name="ctp", bufs=8) as ctp, \
                 tc.tile_pool(name="rop", bufs=6) as rop, \
                 tc.tile_pool(name="evp", bufs=6) as evp, \
                 tc.tile_pool(name="evv", bufs=4) as evv, \
                 tc.tile_pool(name="rtmp", bufs=2) as rtmp, \
                 tc.tile_pool(name="csp", bufs=2) as csp, \
                 tc.tile_pool(name="psA", bufs=6, space="PSUM") as psA:

                # resident weights (one batched DMA per 128-col tile)
                wdkv_ts = load_wcols(wres, wdkv, D, DC, "dkv")     # 4 x [P,16,P]
                wkr_ts = load_wcols(wres, wkr, D, DQR, "kr")       # 2 x [P,16,P]
                wuk_ts = load_wcols(wres, wuk, DC, DQB, "uk")      # 4 x [P,4,P]
                weff_ts = load_wcols(wres, weff, D, DQ, "eff")     # 6 x [P,16,P]
                # W_U_V moving-layout k-tiles [P, DQB]
                wuv_ts = []
                for k in range(KC):
                    wuvt = wres.tile([P, DQB], F32R, name=f"wuvt{k}", tag=f"wuv{k}")
                    nc.sync.dma_start(out=wuvt[:], in_=wuv[k * P:(k + 1) * P, :])
                    wuv_ts.append(wuvt)

                for ch in range(NCH):
                    tsl = slice(ch * CW, (ch + 1) * CW)

                    xts = []
                    for k in range(KD):
                        xt = xp.tile([P, CW], F32R, name="xt", tag="xt")
                        nc.sync.dma_start(out=xt[:], in_=xT[k * P:(k + 1) * P, tsl])
                        xts.append(xt)

                    # c^T slab (DC x CW), kept f32r for kb/v matmuls
                    cts = []
                    for m in range(KC):
                        ct = ctp.tile([P, CW], F32R, name="ct", tag="ct")
                        ps = psA.tile([P, CW], F32, name="ps_c", tag="psa")
                        mm_acc(ps, wdkv_ts[m], xts, KD)
                        nc.any.tensor_copy(ct[:], ps[:])
                        cts.append(ct)

                    # k_base^T (DQB x CW) -> bf16 spill
                    for m in range(DQB // P):
                        ps = psA.tile([P, CW], F32, name="ps_kb", tag="psa")
                        mm_acc(ps, wuk_ts[m], cts, KC)
                        kbt = evp.tile([P, CW], BF16, name="kbt", tag="ev")
                        nc.any.tensor_copy(kbt[:], ps[:])
                        nc.sync.dma_start(out=kbT_d[m * P:(m + 1) * P, tsl], in_=kbt[:])

                    # v natural (CW tokens x DQB) -> bf16 spill
                    for lt in range(CW // P):
                        ps = psA.tile([P, DQB], F32, name="ps_v", tag="psa")
                        for k in range(KC):
                            nc.tensor.matmul(
                                ps[:], cts[k][:, lt * P:(lt + 1) * P], wuv_ts[k][:],
                                start=(k == 0), stop=(k == KC - 1))
                        vt = evv.tile([P, DQB], BF16, name="vt", tag="evv")
                        nc.any.tensor_copy(vt[:], ps[:])
                        nc.sync.dma_start(
                            out=v_d[ch * CW + lt * P: ch * CW + (lt + 1) * P, :],
                            in_=vt[:])

                    # k_rope^T raw (DQR x CW) — held for RoPE below
                    krts = []
                    for m in range(DQR // P):
                        krt = rop.tile([P, CW], F32R, name="krt", tag="rop")
                        ps = psA.tile([P, CW], F32, name="ps_kr", tag="psa")
                        mm_acc(ps, wkr_ts[m], xts, KD)
                        nc.any.tensor_copy(krt[:], ps[:])
                        krts.append(krt)

                    # folded q^T (DQ x CW): m 0..3 = base -> spill, 4..5 = rope raw
                    qrts = []
                    for m in range(DQ // P):
                        ps = psA.tile([P, CW], F32, name="ps_q", tag="psa")
                        mm_acc(ps, weff_ts[m], xts, KD)
                        if m < DQB // P:
                            qbt = evp.tile([P, CW], BF16, name="qbt", tag="ev")
                            nc.any.tensor_copy(qbt[:], ps[:])
                            nc.sync.dma_start(out=qbT_d[m * P:(m + 1) * P, tsl],
                                              in_=qbt[:])
                        else:
                            qrt = rop.tile([P, CW], F32R, name="qrt", tag="rop")
                            nc.any.tensor_copy(qrt[:], ps[:])
                            qrts.append(qrt)

                    # RoPE: final = cos (.) raw + sin (.) (Prot @ raw) -> bf16
                    cos_t = csp.tile([P, CW], F32, name="cos_t", tag="cos")
                    nc.sync.dma_start(out=cos_t[:], in_=cosr[:, tsl])
                    sin_t = csp.tile([P, CW], F32, name="sin_t", tag="sin")
                    nc.sync.dma_start(out=sin_t[:], in_=sinr[:, tsl])
                    for raws, dst in ((qrts, qrT_d), (krts, krT_d)):
                        for m, raw in enumerate(raws):
                            rps = psA.tile([P, CW], F32, name="rps", tag="rps", bufs=2)
                            nc.tensor.matmul(rps[:], prot_t[:], raw[:],
                                             start=True, stop=True)
                            t1 = rtmp.tile([P, CW], F32, name="t1", tag="t1")
                            nc.any.tensor_mul(t1[:], cos_t[:], raw[:])
                            t2 = rtmp.tile([P, CW], F32, name="t2", tag="t2")
                            nc.any.tensor_mul(t2[:], sin_t[:], rps[:])
                            fin = evp.tile([P, CW], BF16, name="fin", tag="ev")
                            nc.any.tensor_add(fin[:], t1[:], t2[:])
                            nc.sync.dma_start(out=dst[m * P:(m + 1) * P, tsl],
                                              in_=fin[:])

            # ================= Phase B: attention (bf16) ====================
            LQ = 512
            with tc.tile_pool(name="khp", bufs=2) as khp, \
                 tc.tile_pool(name="vhp", bufs=2) as vhp, \
                 tc.tile_pool(name="qlq", bufs=3) as qlqp, \
                 tc.tile_pool(name="ptp", bufs=4) as ptp, \
                 tc.tile_pool(name="pap", bufs=2) as pap, \
                 tc.tile_pool(name="rcp", bufs=2) as rcp, \
                 tc.tile_pool(name="stp", bufs=3, space="PSUM") as stp, \
                 tc.tile_pool(name="otp", bufs=2, space="PSUM") as otp, \
                 tc.tile_pool(name="rsp", bufs=2, space="PSUM") as rsp:
                for h in range(NHG):
                    kb_h = khp.tile([P, L], BF16, name="kb_h", tag="kb")
                    nc.sync.dma_start(out=kb_h[:], in_=kbT_d[h * P:(h + 1) * P, :])
                    kr_h = khp.tile([DHR, L], BF16, name="kr_h", tag="kr")
                    nc.sync.dma_start(out=kr_h[:], in_=krT_d[h * DHR:(h + 1) * DHR, :])
                    # all 16 (128x128) V k-tiles for this head in one DMA
                    v_h = vhp.tile([P, L // P, P], BF16, name="v_h", tag="vh")
                    nc.sync.dma_start(
                        out=v_h[:],
                        in_=v_d[:, h * DH:(h + 1) * DH].rearrange(
                            "(lk p) j -> p lk j", p=P))
                    vts = [v_h[:, lk, :] for lk in range(L // P)]
                    for lq in range(L // LQ):
                        qsl = slice(lq * LQ, (lq + 1) * LQ)
                        qb_lq = qlqp.tile([P, LQ], BF16, name="qb_lq", tag="qb")
                        nc.sync.dma_start(out=qb_lq[:],
                                          in_=qbT_d[h * P:(h + 1) * P, qsl])
                        qr_lq = qlqp.tile([DHR, LQ], BF16, name="qr_lq", tag="qr")
                        nc.sync.dma_start(out=qr_lq[:],
                                          in_=qrT_d[h * DHR:(h + 1) * DHR, qsl])

                        ot_ps = otp.tile([P, LQ], F32, name="ot_ps", tag="ot")
                        pt_acc = pap.tile([P, LQ], F32R, name="pt_acc", tag="pa")
                        for lk in range(L // P):
                            st_ps = stp.tile([P, LQ], F32, name="st_ps", tag="st")
                            nc.tensor.matmul(
                                st_ps[:], kb_h[:, lk * P:(lk + 1) * P], qb_lq[:],
                                start=True, stop=False)
                            nc.tensor.matmul(
                                st_ps[:], kr_h[:, lk * P:(lk + 1) * P], qr_lq[:],
                                start=False, stop=True)
                            pt = ptp.tile([P, LQ], BF16, name="pt", tag="pt")
                            nc.scalar.activation(
                                pt[:], st_ps[:], mybir.ActivationFunctionType.Exp,
                                scale=SCALE)
                            nc.tensor.matmul(
                                ot_ps[:], vts[lk][:], pt[:],
                                start=(lk == 0), stop=(lk == L // P - 1))
                            if lk == 0:
                                nc.any.tensor_copy(pt_acc[:], pt[:])
                            else:
                                nc.any.tensor_add(pt_acc[:], pt_acc[:], pt[:])
                        rs_ps = rsp.tile([P, LQ], F32, name="rs_ps", tag="rs")
                        nc.tensor.matmul(rs_ps[:], ones_t[:], pt_acc[:],
                                         start=True, stop=True)
                        rec = rcp.tile([P, LQ], F32, name="rec", tag="rec")
                        nc.vector.reciprocal(rec[:], rs_ps[:])
                        nc.any.tensor_mul(oT_res[h][:, qsl], ot_ps[:], rec[:])

            # ================= Phase C: output projection ===================
            with tc.tile_pool(name="wop", bufs=4) as wop, \
                 tc.tile_pool(name="ocp", bufs=6) as ocp, \
                 tc.tile_pool(name="psC", bufs=4, space="PSUM") as psC:
                wots = []
                for k in range(NHG):
                    wot = wop.tile([P, D], F32R, name="wot", tag="wo")
                    nc.sync.dma_start(out=wot[:], in_=wo[k * P:(k + 1) * P, :])
                    wots.append(wot)
                for mt in range(L // P):
                    for nt in range(D // 512):
                        ps = psC.tile([P, 512], F32, name="ps_o", tag="psc")
                        for k in range(NHG):
                            nc.tensor.matmul(
                                ps[:], oT_res[k][:, mt * P:(mt + 1) * P],
                                wots[k][:, nt * 512:(nt + 1) * 512],
                                start=(k == 0), stop=(k == NHG - 1))
                        oc = ocp.tile([P, 512], F32, name="oc", tag="oc")
                        nc.any.tensor_copy(oc[:], ps[:])
                        nc.sync.dma_start(
                            out=out[mt * P:(mt + 1) * P, nt * 512:(nt + 1) * 512],
                            in_=oc[:])

    nc.compile()
    return nc


def _rope_tables():
    """cos/sin in transposed, 2-head-replicated layout (128 x L), plus Prot^T."""
    inv_freq = 1.0 / (ROPE_THETA ** (np.arange(0, DHR, 2, dtype=np.float32) / DHR))
    ang = np.arange(L, dtype=np.float32)[:, None] * inv_freq[None, :]  # (L, 32)
    cos64 = np.concatenate([np.cos(ang), np.cos(ang)], axis=1).T  # (64, L)
    sin64 = np.concatenate([np.sin(ang), np.sin(ang)], axis=1).T
    cosr = np.ascontiguousarray(np.tile(cos64, (2, 1)), dtype=np.float32)
    sinr = np.ascontiguousarray(np.tile(sin64, (2, 1)), dtype=np.float32)
    # rot(x) = [-x2, x1] per 64-dim head: Prot rows 0:32 = -I at cols 32:64,
    # rows 32:64 = +I at cols 0:32; block-diag over 2 heads; pass transposed.
    p64 = np.zeros((DHR, DHR), dtype=np.float32)
    half = DHR // 2
    p64[np.arange(half), np.arange(half) + half] = -1.0
    p64[np.arange(half) + half, np.arange(half)] = 1.0
    p128 = np.zeros((P, P), dtype=np.float32)
    p128[:DHR, :DHR] = p64
    p128[DHR:, DHR:] = p64
    protT = np.ascontiguousarray(p128.T)
    return cosr, sinr, protT


def _make_in_maps(inputs):
    """Build the 8 per-core input maps from the full-problem input dict."""
    cosr, sinr, protT = _rope_tables()
    f = np.float32
    x = np.asarray(inputs["x"])
    xTs = [np.ascontiguousarray(x[b].T, dtype=f) for b in range(B)]
    # host-side query-path fold (float64 for a clean compose, cast to f32)
    wdq = np.asarray(inputs["W_D_Q"], np.float64)
    weffb_all = (wdq @ np.asarray(inputs["W_U_Q"], np.float64)).astype(f)
    weffr_all = (wdq @ np.asarray(inputs["W_Q_R"], np.float64)).astype(f)
    W_D_KV = np.ascontiguousarray(inputs["W_D_KV"], dtype=f)
    in_maps = []
    for c in range(8):
        b, g = c // 4, c % 4
        hb = slice(g * DQB, (g + 1) * DQB)
        hr = slice(g * DQR, (g + 1) * DQR)
        weff = np.concatenate([weffb_all[:, hb], weffr_all[:, hr]], axis=1)
        in_maps.append(dict(
            xT=xTs[b],
            weff=np.ascontiguousarray(weff, dtype=f),
            wdkv=W_D_KV,
            wuk=np.ascontiguousarray(np.asarray(inputs["W_U_K"])[:, hb], dtype=f),
            wkr=np.ascontiguousarray(np.asarray(inputs["W_K_R"])[:, hr], dtype=f),
            wuv=np.ascontiguousarray(np.asarray(inputs["W_U_V"])[:, hb], dtype=f),
            wo=np.ascontiguousarray(np.asarray(inputs["W_O"])[hb, :], dtype=f),
            cosr=cosr, sinr=sinr, protT=protT,
            onesd=np.ones((P, P), dtype=f),
        ))
    return in_maps


def kernel(x, W_D_Q, W_U_Q, W_Q_R, W_D_KV, W_U_K, W_K_R, W_U_V, W_O):
    if "nc" not in _CACHED:
        _CACHED["nc"] = _build()
    nc = _CACHED["nc"]

    in_maps = _make_in_maps(dict(
        x=x, W_D_Q=W_D_Q, W_U_Q=W_U_Q, W_Q_R=W_Q_R, W_D_KV=W_D_KV,
        W_U_K=W_U_K, W_K_R=W_K_R, W_U_V=W_U_V, W_O=W_O))
    res = run_bass_kernel_spmd(nc, in_maps, core_ids=list(range(8)))
    outs = [r["out"] for r in res.results]
    full = np.stack(
        [outs[b * 4] + outs[b * 4 + 1] + outs[b * 4 + 2] + outs[b * 4 + 3]
         for b in range(B)]).astype(np.float32)
    return full
